# revision 87
# baseline (speedup 1.0000x reference)
# Multi-head attention layer on 8 TRN2 NeuronCores (SPMD, no collectives).
#
# Problem: B=4, N=2048, D=512, H=8 heads (DK=64).
#   out = softmax((q@Wq+bq)(k@Wk+bk)^T / 8) (v@Wv+bv) @ Wo + bo   per (batch, head)
#
# Sharding: core c handles batch b=c//2 and query-row half c%2 (1024 rows).
# K/V projections are recomputed by both cores of a pair (cheap) so there is
# no cross-core communication at all.
#
# Per-core dataflow (all layouts chosen so NO on-chip transposes are needed;
# the host pre-transposes inputs to (channel, token) layout and casts bf16):
#   K^T(d,k)  = Wk-chunks.T @ kT          (PE), +bias via DVE add
#   Q^T(d,q)  = Wq-chunks.T @ qT          (PE), +bias via DVE add
#   V(k,d)    = vT-chunks.T @ Wv          (PE), +bias via DVE add of a
#               DMA-broadcast bias tile; per (kt, head) a 128-wide block
#               [V(64)|ones(64)] (even heads) / [ones|V] (odd heads)
#   S^T(k,q)  = K^T_tile.T @ Q^T  per head  (PE, contraction d=64)
#   P^T       = exp(S^T/8)                (ACT, scale folded into activation)
#   ctx^T(d,q)= [V|ones]-block.T @ P^T    (PE, accumulated over k; the ones
#               half replicates the softmax denominator onto the 64
#               partitions opposite the ctx rows)
#   norm      = ctx^T * reciprocal(denominator)  (DVE reciprocal_approx_fast
#               + one partition-shift SBUF->SBUF DMA + DVE multiply;
#               head 7's normalize folds into the output projection as a
#               per-partition scalar instead)
#   out(n,d)  = ctxn-chunks.T @ Wo + bo   (PE, bias via rank-1 matmul)
#
# Scheduling (the ACT engine's exp stream, 128 x 1.11us, is the binding
# resource; the whole kernel is organized to keep it saturated):
#   - global software pipeline over (head, kt): S/exp lead, AV lags ~5 tiles
#     and drains into the next head's early kt slots (no boundary lumps)
#   - 3 PSUM st buffers so S(kt+2) never waits on exp(kt); PE stays dense,
#     which also keeps the HAM clock gate at 8/8 (2.4 GHz)
#   - K/Q/V projections trickle in as ~0.9-1.7us chunks at fixed kt slots;
#     DMA issue order matches first-use order so head 0 starts ~10us in
#   - heads 6/7 carry zero-contribution filler matmuls: without PE filler
#     the exp-wait micro-idles re-throttle the PE to 1.2 GHz mid-kernel
from contextlib import ExitStack

import numpy as np
import ml_dtypes

import concourse.bass as bass
import concourse.mybir as mybir
import concourse.tile as tile
from concourse import bacc
from concourse.bass_utils import run_bass_kernel_spmd

BF16 = mybir.dt.bfloat16
F32 = mybir.dt.float32
Exp = mybir.ActivationFunctionType.Exp

B, N, D, H = 4, 2048, 512, 8
DK = D // H          # 64
NQ = N // 2          # 1024 query rows per core
NKT = N // 128       # 16 k tiles


def build_nc():
    nc = bacc.Bacc("TRN2", target_bir_lowering=False)

    qT = nc.dram_tensor("qT", (D, NQ), BF16, kind="ExternalInput")
    kT = nc.dram_tensor("kT", (D, N), BF16, kind="ExternalInput")
    vT = nc.dram_tensor("vT", (D, N), BF16, kind="ExternalInput")
    wq = nc.dram_tensor("wq", (D, D), BF16, kind="ExternalInput")
    wk = nc.dram_tensor("wk", (D, D), BF16, kind="ExternalInput")
    wv = nc.dram_tensor("wv", (D, D), BF16, kind="ExternalInput")
    wo = nc.dram_tensor("wo", (D, D), BF16, kind="ExternalInput")
    bq = nc.dram_tensor("bq", (D, 1), F32, kind="ExternalInput")
    bk = nc.dram_tensor("bk", (D, 1), F32, kind="ExternalInput")
    bv = nc.dram_tensor("bv", (1, D), BF16, kind="ExternalInput")
    bo = nc.dram_tensor("bo", (1, D), BF16, kind="ExternalInput")
    out = nc.dram_tensor("out", (NQ, D), BF16, kind="ExternalOutput")

    with tile.TileContext(nc) as tc:
        with ExitStack() as ctx:
            emit(ctx, tc, qT, kT, vT, wq, wk, wv, wo, bq, bk, bv, bo, out)
    nc.compile()
    return nc


def emit(ctx, tc, qT, kT, vT, wq, wk, wv, wo, bq, bk, bv, bo, out):
    nc = tc.nc
    consts = ctx.enter_context(tc.tile_pool(name="consts", bufs=1))
    p_pool = ctx.enter_context(tc.tile_pool(name="p_pool", bufs=8))
    post = ctx.enter_context(tc.tile_pool(name="post", bufs=2))
    outs = ctx.enter_context(tc.tile_pool(name="outs", bufs=6))
    # 6 PSUM banks for the S/projection scratch ring (3 x 2-bank tiles, deep
    # enough that S(kt+2) never waits on exp(kt)), 2 banks for ctx
    s_pool = ctx.enter_context(tc.tile_pool(name="s_pool", bufs=3, space="PSUM"))
    c_pool = ctx.enter_context(tc.tile_pool(name="c_pool", bufs=1, space="PSUM"))
    dram = ctx.enter_context(tc.tile_pool(name="dram", bufs=1, space="DRAM"))

    # ---- inputs (DMA order = first-use order; big tensors in halves) -------
    def load(name, shape, dt_, src_ap, eng=None):
        t = consts.tile(shape, dt_, name=name)
        (eng or nc.sync).dma_start(out=t, in_=src_ap)
        return t

    def load_halves(name, shape, dt_, dram_t, n, parts=2):
        t = consts.tile(shape, dt_, name=name)
        h = n // parts
        for i in range(parts):
            nc.sync.dma_start(
                out=t[:, :, i * h:(i + 1) * h],
                in_=dram_t[:, i * h:(i + 1) * h].rearrange(
                    "(c p) n -> p c n", p=128))
        return t

    def load_part(t, dram_t, n0, n1):
        nc.sync.dma_start(
            out=t[:, :, n0:n1],
            in_=dram_t[:, n0:n1].rearrange("(c p) n -> p c n", p=128))

    # DMA issue order = critical-path order: everything the first S matmul
    # needs (wk, kT half 0, bk, wq, qT, bq) streams first; kT's second half
    # and all of V arrive while head 0 is already running.
    wk_s = load("wk_s", [128, 4, D], BF16, wk[:].rearrange("(c p) d -> p c d", p=128))
    kT_s = consts.tile([128, 4, N], BF16, name="kT_s")
    load_part(kT_s, kT, 0, 512)
    load_part(kT_s, kT, 512, 1024)
    bk_s = load("bk_s", [128, 4, 1], F32, bk[:].rearrange("(c p) o -> p c o", p=128))
    wq_s = load("wq_s", [128, 4, D], BF16, wq[:].rearrange("(c p) d -> p c d", p=128))
    qT_s = load_halves("qT_s", [128, 4, NQ], BF16, qT, NQ)
    bq_s = load("bq_s", [128, 4, 1], F32, bq[:].rearrange("(c p) o -> p c o", p=128))
    wv_s = load("wv_s", [128, 4, D], BF16, wv[:].rearrange("(c p) d -> p c d", p=128))
    bv_bc = load("bv_bc", [128, D], BF16, bv[:].to_broadcast((128, D)))
    load_part(kT_s, kT, 1024, 1536)
    load_part(kT_s, kT, 1536, 2048)
    vT_s = load_halves("vT_s", [128, 4, N], BF16, vT, N, parts=4)
    wo_s = load("wo_s", [128, 4, D], BF16, wo[:].rearrange("(c p) d -> p c d", p=128))
    bo_s = load("bo_s", [1, D], BF16, bo[:])

    ones1 = consts.tile([1, 128], BF16)
    nc.vector.memset(ones1, 1.0)
    ztile = consts.tile([128, 512], BF16)
    nc.vector.memset(ztile, 0.0)

    # tiny dummy exp: pulls the ~2.7us ACT_TABLE_LOAD for the exp set into
    # the initial DMA window instead of the first real exp
    tl = consts.tile([128, 16], F32)
    nc.scalar.activation(tl, ztile[:, 0:16], Exp, scale=1.0)

    KT_s = consts.tile([128, 4, N], BF16)     # K^T, d on partitions
    QT_s = consts.tile([128, 4, NQ], BF16)    # Q^T, d on partitions
    # V with k on partitions; per (kt, head) a 128-wide stationary block:
    # even heads [V(64) | ones(64)], odd heads [ones(64) | V(64)].  The ones
    # half replicates the softmax denominator onto the 64 partitions opposite
    # the ctx rows, so normalization needs no partition broadcast.
    V_s = consts.tile([128, NKT, H, 128], BF16)
    ctxn_s = consts.tile([128, 4, NQ], BF16)  # normalized ctx^T, dmid on partitions

    V_pairs = V_s[:].rearrange("p t (j par) w -> p t par j w", par=2)
    nc.vector.memset(V_pairs[:, :, 0, :, 64:128], 1.0)  # even heads: ones right
    nc.vector.memset(V_pairs[:, :, 1, :, 0:64], 1.0)    # odd heads: ones left

    # ---- projections --------------------------------------------------------
    def emit_kproj_half(dt, kh):  # one kT half: 8 MMs, one st ring slot
        st = s_pool.tile([128, 1024], F32, tag="s", name="st_k")
        for kc in range(2):
            for cc in range(4):
                nc.tensor.matmul(
                    st[:, kc * 512:(kc + 1) * 512],
                    lhsT=wk_s[:, cc, dt * 128:(dt + 1) * 128],
                    rhs=kT_s[:, cc, kh * 1024 + kc * 512:
                             kh * 1024 + (kc + 1) * 512],
                    start=(cc == 0), stop=(cc == 3))
        nc.vector.tensor_scalar_add(
            KT_s[:, dt, kh * 1024:(kh + 1) * 1024], st, bk_s[:, dt, :])

    def emit_qproj(dt):  # 8 MMs, one st ring slot
        st = s_pool.tile([128, 1024], F32, tag="s", name="st_q")
        for qc in range(2):
            for cc in range(4):
                nc.tensor.matmul(
                    st[:, qc * 512:(qc + 1) * 512],
                    lhsT=wq_s[:, cc, dt * 128:(dt + 1) * 128],
                    rhs=qT_s[:, cc, qc * 512:(qc + 1) * 512],
                    start=(cc == 0), stop=(cc == 3))
        nc.vector.tensor_scalar_add(QT_s[:, dt, :], st, bq_s[:, dt, :])

    def emit_vproj_pair(g):  # V projection for k tiles 2g, 2g+1 (one st tile)
        st = s_pool.tile([128, 1024], F32, tag="s", name="st_v")
        for sub in range(2):
            kt = 2 * g + sub
            sl = st[:, sub * 512:(sub + 1) * 512]
            for cc in range(4):
                nc.tensor.matmul(
                    sl,
                    lhsT=vT_s[:, cc, kt * 128:(kt + 1) * 128],
                    rhs=wv_s[:, cc, :],
                    start=(cc == 0), stop=(cc == 3))
            # scatter per-head 64-col blocks: even heads to cols 0:64 of
            # their V_s slot, odd heads to cols 64:128
            sl_pairs = sl.rearrange("p (j par w) -> p par j w", par=2, w=64)
            bv_pairs = bv_bc[:].rearrange("p (j par w) -> p par j w",
                                          par=2, w=64)
            vt_pairs = V_s[:, kt].rearrange("p (j par) w -> p par j w", par=2)
            nc.vector.tensor_add(
                vt_pairs[:, 0, :, 0:64], sl_pairs[:, 0], bv_pairs[:, 0])
            nc.vector.tensor_add(
                vt_pairs[:, 1, :, 64:128], sl_pairs[:, 1], bv_pairs[:, 1])

    # ---- attention ----------------------------------------------------------
    def emit_s_exp(h, kt):
        """S^T tile for (head, kt) on PE, then exp on ACT. Returns pt."""
        dt = h // 2
        kt_lhs = KT_s[(h % 2) * 64:(h % 2) * 64 + 64, dt, :]
        q_rhs = QT_s[(h % 2) * 64:(h % 2) * 64 + 64, dt, :]
        st = s_pool.tile([128, 1024], F32, tag="s", name="st_s")
        for qc in range(2):
            nc.tensor.matmul(
                st[:, qc * 512:(qc + 1) * 512],
                lhsT=kt_lhs[:, kt * 128:(kt + 1) * 128],
                rhs=q_rhs[:, qc * 512:(qc + 1) * 512],
                start=True, stop=True)
        pt = p_pool.tile([128, 1024], BF16, tag="p", name="pt")
        nc.scalar.activation(pt, st, Exp, scale=0.125)
        return pt

    def emit_av(h, kt, pt, ctx_ps):
        # [V|ones] (even h) / [ones|V] (odd h): ctx rows on one 64-partition
        # half, the softmax denominator replicated on the other half
        for qc in range(2):
            nc.tensor.matmul(
                ctx_ps[:, qc * 512:(qc + 1) * 512],
                lhsT=V_s[:, kt, h, :],
                rhs=pt[:, qc * 512:(qc + 1) * 512],
                start=(kt == 0), stop=(kt == NKT - 1))

    # ---- per-head normalize helpers ----------------------------------------
    # normalize: ctx rows sit on one 64-partition half of ctx_ps, the
    # denominator (replicated 64x by the ones block) on the other half.
    # Both halves are copied to SBUF promptly so ctx_ps (the single c_pool
    # buffer) frees fast for the next head.  reciprocal_approx_fast only
    # works at base partition 0 and cannot read PSUM, so it runs on
    # partitions 0-63 of the SBUF copy, with the partition-shifting
    # SBUF->SBUF DMA on whichever side needs it.  The reciprocal/DMA/multiply
    # chain is deferred to a quiet DVE-FIFO slot in the next head.
    norm7 = {}   # h7 leftovers for the output projection finish pass

    def emit_norm_copies(h, ctx_ps):
        dt, even = h // 2, (h % 2 == 0)
        cl, dl = (0, 64) if even else (64, 0)   # ctx / denom partition bases
        den = post.tile([128, NQ], F32, tag="den", name="den")
        nc.vector.tensor_copy(out=den[dl:dl + 64, :], in_=ctx_ps[dl:dl + 64, :])
        if h == H - 1:
            # last head (odd: ctx on 64:128, denom on 0:63): its normalize
            # folds into the output projection instead - stage the raw ctx
            # to SBUF bf16 (the matmul lhsT) and later scale the h7 partial
            # product by recip[n] as a per-partition scalar. No partition
            # shift DMA, no multiply on the critical tail.
            ctxc = post.tile([128, NQ], BF16, tag="ctxc7", name="ctxc7")
            nc.vector.tensor_copy(out=ctxc[64:128, :], in_=ctx_ps[64:128, :])
            norm7["ctxc"] = ctxc

            def tail():
                rc = post.tile([128, NQ], F32, tag="rc", name="rc")
                nc.vector.reciprocal_approx_fast(out=rc[0:64, :],
                                                 in_=den[0:64, :])
                # transpose the 1024 reciprocals onto partitions
                # (rcol[p, f] = rc[0, f*128 + p]) via a 4KB DRAM bounce -
                # the partition-from-free scatter needs a DRAM-side AP
                dr = dram.tile([1, NQ], F32, tag="dr", name="dr")
                nc.sync.dma_start(out=dr, in_=rc[0:1, :])
                rcol = post.tile([128, 8], F32, tag="rcol", name="rcol")
                nc.sync.dma_start(
                    out=rcol,
                    in_=dr[:].rearrange("o (f p) -> (o p) f", p=128))
                norm7["rcol"] = rcol
            return tail

        ctxc = post.tile([128, NQ], F32, tag="ctxc", name="ctxc")
        nc.vector.tensor_copy(out=ctxc[cl:cl + 64, :],
                              in_=ctx_ps[cl:cl + 64, :])

        def tail():
            d = den
            if dl != 0:
                den2 = post.tile([128, NQ], F32, tag="den2", name="den2")
                nc.sync.dma_start(out=den2[0:64, :], in_=den[dl:dl + 64, :])
                d = den2
            rc = post.tile([128, NQ], F32, tag="rc", name="rc")
            nc.vector.reciprocal_approx_fast(out=rc[0:64, :], in_=d[0:64, :])
            if cl != 0:
                rc2 = post.tile([128, NQ], F32, tag="rc2", name="rc2")
                nc.sync.dma_start(out=rc2[cl:cl + 64, :], in_=rc[0:64, :])
                rc = rc2
            nc.vector.tensor_mul(ctxn_s[cl:cl + 64, dt, :],
                                 ctxc[cl:cl + 64, :], rc[cl:cl + 64, :])
        return tail

    # ---- output projection --------------------------------------------------
    # Output projection, split in two phases: the dc 0-2 accumulation only
    # needs heads 0-5 (normalized long ago) so it overlaps the last head's
    # normalize chain; the finish pass adds dc 3 (heads 6/7) + bias, then
    # copies out.  At the tail the PE is the idle engine, so the bias rides a
    # rank-1 matmul, and the PSUM->SBUF copies alternate between the (also
    # idle) ACT engine and the DVE.
    def emit_outproj_main(g):  # n tiles 2g, 2g+1; everything except head 7
        st = s_pool.tile([128, 1024], F32, tag="s", name="st_o")
        for sub in range(2):
            nt = g * 2 + sub
            sl = st[:, sub * 512:(sub + 1) * 512]
            for dc in range(3):
                nc.tensor.matmul(
                    sl,
                    lhsT=ctxn_s[:, dc, nt * 128:(nt + 1) * 128],
                    rhs=wo_s[:, dc, :],
                    start=(dc == 0), stop=False)
            # head 6's half of dc 3 (64-wide contraction): its normalize
            # finished back at head 7 kt12, so only head 7's half remains
            # for the finish pass; the bias also lands here
            nc.tensor.matmul(
                sl,
                lhsT=ctxn_s[0:64, 3, nt * 128:(nt + 1) * 128],
                rhs=wo_s[0:64, 3, :],
                start=False, stop=False)
            nc.tensor.matmul(sl, lhsT=ones1, rhs=bo_s, start=False, stop=True)
        return st

    def emit_outproj_fin(g, st):
        # head 7's (unnormalized) partial product goes to its own PSUM tile,
        # staged to SBUF by the tail-idle ACT engine (frees the 1-deep c-ring
        # fast so groups pipeline), then one fused op merges it:
        # out = h7_partial * recip7[n] + rest.  Merges alternate between the
        # DVE and the idle gpsimd engine (all-SBUF operands).
        st7 = c_pool.tile([128, 1024], F32, tag="c", name="st7")
        for sub in range(2):
            nt = g * 2 + sub
            sl7 = st7[:, sub * 512:(sub + 1) * 512]
            nc.tensor.matmul(
                sl7,
                lhsT=norm7["ctxc"][64:128, nt * 128:(nt + 1) * 128],
                rhs=wo_s[64:128, 3, :],
                start=True, stop=True)
            sb7 = outs.tile([128, D], F32, tag="sb7", name="sb7")
            nc.scalar.copy(out=sb7, in_=sl7)
            ot = outs.tile([128, D], BF16, tag="o", name="ot")
            nc.vector.tensor_copy(out=ot,
                                  in_=st[:, sub * 512:(sub + 1) * 512])
            ot2 = outs.tile([128, D], BF16, tag="o2", name="ot2")
            nc.vector.scalar_tensor_tensor(
                out=ot2, in0=sb7, scalar=norm7["rcol"][:, nt:nt + 1], in1=ot,
                op0=mybir.AluOpType.mult, op1=mybir.AluOpType.add)
            nc.sync.dma_start(out=out[nt * 128:(nt + 1) * 128, :], in_=ot2)

    # ---- schedule -----------------------------------------------------------
    # warm-up: dummy matmuls during the initial DMA window push the PE's HAM
    # clock gate to 8/8 before the first real projection matmuls issue
    wst = s_pool.tile([128, 1024], F32, tag="s", name="wst")
    for i in range(20):
        nc.tensor.matmul(wst[:, 0:512], lhsT=ztile[:, 0:128], rhs=ztile,
                         start=(i == 0), stop=(i == 19))
    emit_kproj_half(0, 0)
    emit_qproj(0)
    # Projection prefetch is interleaved mid-head in ~0.9us chunks: each is
    # small enough for the exp pipeline's buffered ACT work to absorb, the
    # DVE bias adds land while the DVE FIFO is empty, and the steady trickle
    # of PE work keeps the HAM clock gate warm.
    def head_mid(h):
        dt = h // 2
        if h == 0:
            # kproj's second half and the V projection (their kT/vT quarters
            # arrive during head 0) trickle in as their HBM data lands, each
            # pair just before AV (delayed 5 kt) needs its k-tiles
            mid = {3: lambda: emit_kproj_half(0, 1),
                   15: lambda: emit_vproj_pair(6)}
            for g in range(6):
                mid[4 + 2 * g] = (lambda g=g: emit_vproj_pair(g))
            return mid
        if h == 1:
            return {0: lambda: emit_vproj_pair(7),
                    5: lambda: emit_kproj_half(1, 0),
                    9: lambda: emit_kproj_half(1, 1),
                    13: lambda: emit_qproj(1)}
        if h % 2 == 0 and dt < 3:
            return {5: lambda: emit_kproj_half(dt + 1, 0),
                    11: lambda: emit_kproj_half(dt + 1, 1)}
        if h % 2 == 1 and dt < 3:
            return {8: lambda: emit_qproj(dt + 1)}
        return {}

    # Global software pipeline across all (head, kt): AV lags S/exp by ~5
    # tiles and drains INTO the next head's early kt slots, so no exp ever
    # waits behind a lump of trailing AV matmuls at a head boundary.
    ctxs = {}
    tails = {}
    pending = []   # (h, kt, pt) with AV not yet emitted

    def pop_av():
        ph, pkt, ppt = pending.pop(0)
        emit_av(ph, pkt, ppt, ctxs[ph])
        if pkt == NKT - 1:
            tails[ph] = emit_norm_copies(ph, ctxs[ph])

    for h in range(H):
        mid = head_mid(h)
        filler = not mid
        ctxs[h] = c_pool.tile([128, 1024], F32, tag="c", name="ctx_ps")
        for kt in range(NKT):
            if kt in mid:
                mid[kt]()
            if kt == 12 and h >= 1:
                tails.pop(h - 1)()   # deferred recip/shift/mul of prev head
            if filler and kt >= 5:
                nc.tensor.matmul(ctxs[h][:, 0:512], lhsT=ztile[:, 0:128],
                                 rhs=ztile, start=False, stop=False,
                                 skip_group_check=True)
            pt = emit_s_exp(h, kt)
            pending.append((h, kt, pt))
            if h == 7 and kt >= 10:
                target = max(2, 5 - (kt - 9) // 2)
            elif kt <= 1:
                target = 4 - kt     # drain prev head's AVs fast
            elif kt == 2:
                target = 3
            else:
                target = 5
            while len(pending) > target:
                pop_av()
    while pending:
        pop_av()
    tails.pop(7)()
    sts = [emit_outproj_main(g) for g in range(3)]
    emit_outproj_fin(0, sts[0])
    sts.append(emit_outproj_main(3))
    for g in range(1, 4):
        emit_outproj_fin(g, sts[g])


_NC_CACHE = None


def _get_nc():
    global _NC_CACHE
    if _NC_CACHE is None:
        _NC_CACHE = build_nc()
    return _NC_CACHE


def make_in_maps(query, key, value, Wq, bq, Wk, bk, Wv, bv, Wo, bo):
    bf = ml_dtypes.bfloat16
    f = np.float32
    query = np.asarray(query, f)
    key = np.asarray(key, f)
    value = np.asarray(value, f)
    shared = {
        "wq": np.asarray(Wq, f).astype(bf),
        "wk": np.asarray(Wk, f).astype(bf),
        "wv": np.asarray(Wv, f).astype(bf),
        "wo": np.asarray(Wo, f).astype(bf),
        "bq": np.asarray(bq, f).reshape(D, 1),
        "bk": np.asarray(bk, f).reshape(D, 1),
        "bv": np.asarray(bv, f).astype(bf).reshape(1, D),
        "bo": np.asarray(bo, f).astype(bf).reshape(1, D),
    }
    kTs = [np.ascontiguousarray(key[b].T).astype(bf) for b in range(B)]
    vTs = [np.ascontiguousarray(value[b].T).astype(bf) for b in range(B)]
    in_maps = []
    for c in range(8):
        b, half = c // 2, c % 2
        m = dict(shared)
        m["qT"] = np.ascontiguousarray(
            query[b, half * NQ:(half + 1) * NQ, :].T).astype(bf)
        m["kT"] = kTs[b]
        m["vT"] = vTs[b]
        in_maps.append(m)
    return in_maps


def run(inputs, trace=False):
    nc = _get_nc()
    in_maps = make_in_maps(**inputs)
    res = run_bass_kernel_spmd(nc, in_maps, core_ids=list(range(8)), trace=trace)
    out = np.empty((B, N, D), np.float32)
    for c in range(8):
        b, half = c // 2, c % 2
        out[b, half * NQ:(half + 1) * NQ, :] = np.asarray(
            res.results[c]["out"], dtype=np.float32)
    return out, res


def kernel(**inputs):
    out, _ = run(inputs, trace=False)
    return out
